# revision 57
# baseline (speedup 1.0000x reference)
"""DCNv3-1D fused Trainium2 kernel (8-core batch-parallel SPMD), v4.

Reference semantics (per batch row, one core per row):
  x_proj = x @ W_in + b_in
  y      = depthwise_conv3(x) + conv_b ; LN over C ; GELU -> x_feat
  offset = x_feat @ W_off + b_off ; mask = softmax_K(x_feat @ W_mask + b_mask)
  loc    = l + dil_grid + offset (mod L); bilinear sample x_proj along L
  out    = (sum_k mask * sampled) @ W_out + b_out

v4 dataflow (vs v3):
 - fp16 everywhere on-chip (10-bit mantissa; floor math carries a host-folded
   +3 bias so truncation == floor with no is_gt fixup).
 - x is pre-transposed on the HOST to [C, 1+L+1] with zero halo cols; no
   on-device input transposes.
 - LN stats math runs directly on the ones-matmul broadcast PSUM rows: no
   second broadcast matmul pair, no rstd/mean evacuations.  rstd via
   exp(-0.5*ln(var+eps)) keeps the scalar engine on one activation table
   (Copy/Identity/Square/Ln/Exp); only GELU switches tables, batched 1x/sc.
 - offset/mask biases (+dil grid, +3) ride the om PSUM evacuation as a
   per-partition activation bias in q-major layout.
 - band weights abf[l, tc, q=s*G+g] (48-row padded) -> xbar transpose ->
   aT[q, l] -> group->channel expansion via replicated SBUF->SBUF DMAs
   (zero matmuls, zero evacuations for the expansion).
 - apply = 10 fp16 DVE mults (z_s = ws_s * xp shifted); the 5-shift
   accumulation is folded into the output projection as PSUM accumulation
   (out = sum_s sum_ci Wout^T @ z_s), eliminating all DVE adds.
 - 2 superchunks of 2048 pipeline feature->band->apply->out across engines.
"""

import numpy as np

import concourse.bacc as bacc
import concourse.bass as bass
import concourse.mybir as mybir
from concourse.tile import TileContext
from concourse.bass_utils import run_bass_kernel_spmd

N, L, C, G, K = 8, 4096, 256, 8, 3
GC = C // G
H = C // 128          # 2 channel halves
NS = 5                # shifts sigma = s-2, s in 0..4
Q = 48                # q = s*G + g rows (40 used, pad to 48 = 3*16 xbar rows)
SC = 2                # superchunks
BCW = L // SC         # 2048
TC = BCW // 128       # 16 l-tiles per superchunk
FCW = 1024            # feature chunk width (psum tile)
LN_EPS = 1e-6
FB = 3.0              # floor bias
XO = 1                # conv zero-halo offset in xT
XHO = 2               # wrap halo offset in xpT

F32 = mybir.dt.float32
F16 = mybir.dt.float16
I32 = mybir.dt.int32
Alu = mybir.AluOpType
Act = mybir.ActivationFunctionType

_CACHE = {}


def _build(flags):
    nc = bacc.Bacc("TRN2", target_bir_lowering=False, debug=False, num_devices=8)

    xt_d = nc.dram_tensor("xt", [C, L + 2], F16, kind="ExternalInput")
    NBF = H * C + H * C + H * Q + K * H * 128 + 128 + H * NS * 128
    cbf = nc.dram_tensor("cbf", [128, NBF], F16, kind="ExternalInput")
    NF3 = 1 + 2 + 3 * H  # bomd, edge(2), bin/convb/bout per-partition cols
    cf3 = nc.dram_tensor("cf3", [128, NF3], F32, kind="ExternalInput")
    if flags["has_ln"]:
        lngb = nc.dram_tensor("lngb", [128, H, 2], F16, kind="ExternalInput")
    out_d = nc.dram_tensor("out", [C, L], F16, kind="ExternalOutput")

    with TileContext(nc) as tc, nc.allow_low_precision(reason="fp16 by design"):
        _emit(nc, tc, flags, locals())
    nc.compile()
    return nc


def _emit(nc, tc, flags, dram):
    from contextlib import ExitStack

    ctx = ExitStack()
    with ctx:
        consts = ctx.enter_context(tc.tile_pool(name="consts", bufs=1))
        big = ctx.enter_context(tc.tile_pool(name="big", bufs=1))
        feat = ctx.enter_context(tc.tile_pool(name="feat", bufs=1))
        statp = ctx.enter_context(tc.tile_pool(name="statp", bufs=1))
        bandp = ctx.enter_context(tc.tile_pool(name="bandp", bufs=1))
        wsp = ctx.enter_context(tc.tile_pool(name="wsp", bufs=1))
        psA = ctx.enter_context(tc.tile_pool(name="psA", bufs=3, space="PSUM"))
        psB = ctx.enter_context(tc.tile_pool(name="psB", bufs=2, space="PSUM"))

        # ---- constants ----
        cb = consts.tile([128, dram["NBF"]], F16, tag="cb", name="cb")
        nc.sync.dma_start(out=cb, in_=dram["cbf"][:])
        o = 0
        c_win = cb[:, o:o + H * C].rearrange("p (h c) -> p h c", h=H); o += H * C
        c_wout = cb[:, o:o + H * C].rearrange("p (h c) -> p h c", h=H); o += H * C
        c_wom = cb[:, o:o + H * Q].rearrange("p (h c) -> p h c", h=H); o += H * Q
        c_dconv = cb[:, o:o + K * H * 128].rearrange(
            "p (k h c) -> p k h c", k=K, h=H); o += K * H * 128
        c_ones = cb[:, o:o + 128]; o += 128
        c_E = cb[:, o:o + H * NS * 128].rearrange(
            "p (i c) -> p i c", c=128); o += H * NS * 128
        cf = consts.tile([128, dram["NF3"]], F32, tag="cf", name="cf")
        nc.sync.dma_start(out=cf, in_=dram["cf3"][:])
        c_bomd = cf[:, 0:1]
        c_edge = cf[:, 1:3]
        c_bin = cf[:, 3:3 + H]
        c_convb = cf[:, 3 + H:3 + 2 * H]
        c_bout = cf[:, 3 + 2 * H:3 + 3 * H]
        c_eps = consts.tile([128, 1], F32, tag="c_eps", name="c_eps")
        nc.vector.memset(c_eps, LN_EPS)
        if flags["has_ln"]:
            c_lngb = consts.tile([128, H, 2], F16, tag="c_lngb", name="c_lngb")
            nc.sync.dma_start(out=c_lngb, in_=dram["lngb"][:])

        # ---- x load (host-pretransposed, fp16, zero halo), 2 slices per
        # half so the first xpT/conv matmuls start at half-load ----
        xv = dram["xt_d"].rearrange("(h p) l -> p h l", p=128)
        xT = []
        MID = L // 2 + 2
        for h in range(H):
            t_ = big.tile([128, L + 2], F16, tag=f"xT{h}", name=f"xT{h}")
            eng = nc.sync if h == 0 else nc.scalar
            eng.dma_start(out=t_[:, 0:MID], in_=xv[:, h, 0:MID])
            eng.dma_start(out=t_[:, MID:L + 2], in_=xv[:, h, MID:L + 2])
            xT.append(t_)

        # ---- x_proj^T (c-major) with 2-col wrap halo; emitted after om(sc0)
        # so the PE starts on the conv->LN->om->band critical chain first and
        # fills its band-wait window with this off-chain work ----
        xpT = []

        def xpt_block():
            for hp in range(H):
                t_ = big.tile([128, XHO + L + 2], F16, tag=f"xpT{hp}",
                              name=f"xpT{hp}")
                for ch in range(L // FCW):
                    ps = psA.tile([128, FCW], F32, tag="ps", name="ps_xp")
                    for q in range(2):
                        for h in range(H):
                            nc.tensor.matmul(
                                ps[:, q * 512:(q + 1) * 512],
                                lhsT=c_win[:, h, hp * 128:(hp + 1) * 128],
                                rhs=xT[h][:, XO + ch * FCW + q * 512:
                                          XO + ch * FCW + (q + 1) * 512],
                                start=(h == 0), stop=(h == H - 1),
                            )
                    dst = t_[:, XHO + ch * FCW:XHO + (ch + 1) * FCW]
                    if flags["has_bin"]:
                        nc.scalar.activation(out=dst, in_=ps, func=Act.Identity,
                                             bias=c_bin[:, hp:hp + 1])
                    else:
                        nc.scalar.activation(out=dst, in_=ps, func=Act.Copy)
                nc.vector.tensor_copy(out=t_[:, 0:XHO], in_=t_[:, L:L + XHO])
                nc.vector.tensor_copy(out=t_[:, XHO + L:XHO + L + 2],
                                      in_=t_[:, XHO:XHO + 2])
                xpT.append(t_)

        omT = big.tile([Q, L], F16, tag="omT", name="omT")

        def feature_chunk(fc):
            """conv -> LN stats -> norm -> GELU for cols [fc*FCW, (fc+1)*FCW)."""
            base = fc * FCW
            ybs, fts = [], []
            for h in range(H):
                ps = psA.tile([128, FCW], F32, tag="ps", name="ps_y")
                for j in range(K):
                    for q in range(2):
                        nc.tensor.matmul(
                            ps[:, q * 512:(q + 1) * 512],
                            lhsT=c_dconv[:, j, h, :],
                            rhs=xT[h][:, XO + base + q * 512 + j - 1:
                                      XO + base + q * 512 + j + 511],
                            start=(j == 0), stop=(j == K - 1),
                        )
                yb = feat.tile([128, FCW], F16, tag="yb", name="yb", bufs=3)
                if flags["has_convb"]:
                    nc.scalar.activation(out=yb, in_=ps, func=Act.Identity,
                                         bias=c_convb[:, h:h + 1])
                else:
                    nc.scalar.activation(out=yb, in_=ps, func=Act.Copy)
                ysq = feat.tile([128, FCW], F16, tag="ysq", name="ysq", bufs=2)
                nc.vector.tensor_tensor(out=ysq, in0=yb, in1=yb, op=Alu.mult)
                ybs.append((yb, ysq))
            psm = psA.tile([128, FCW], F32, tag="ps", name="ps_mu")
            for q in range(2):
                for h in range(H):
                    nc.tensor.matmul(psm[:, q * 512:(q + 1) * 512], lhsT=c_ones,
                                     rhs=ybs[h][0][:, q * 512:(q + 1) * 512],
                                     start=(h == 0), stop=(h == H - 1))
            pss = psA.tile([128, FCW], F32, tag="ps", name="ps_sq")
            for q in range(2):
                for h in range(H):
                    nc.tensor.matmul(pss[:, q * 512:(q + 1) * 512], lhsT=c_ones,
                                     rhs=ybs[h][1][:, q * 512:(q + 1) * 512],
                                     start=(h == 0), stop=(h == H - 1))
            # stats on the broadcast psum rows (all 128 partitions identical)
            psmb = statp.tile([128, FCW], F16, tag="st", name="psmb", bufs=6)
            nc.scalar.activation(out=psmb, in_=psm, func=Act.Copy)
            tsq = statp.tile([128, FCW], F16, tag="st", name="tsq", bufs=6)
            nc.vector.tensor_tensor(out=tsq, in0=psmb, in1=psmb, op=Alu.mult)
            vc = statp.tile([128, FCW], F16, tag="st", name="vc", bufs=6)
            nc.vector.tensor_tensor(out=vc, in0=pss, in1=tsq, op=Alu.subtract)
            lnv = statp.tile([128, FCW], F16, tag="st", name="lnv", bufs=6)
            nc.scalar.activation(out=lnv, in_=vc, func=Act.Ln,
                                 bias=c_eps[:, 0:1])
            rsb = statp.tile([128, FCW], F16, tag="st", name="rsb", bufs=6)
            nc.scalar.activation(out=rsb, in_=lnv, func=Act.Exp, scale=-0.5)
            m2b = statp.tile([128, FCW], F16, tag="st", name="m2b", bufs=6)
            nc.vector.tensor_tensor(out=m2b, in0=psmb, in1=rsb, op=Alu.mult)
            for h in range(H):
                yb = ybs[h][0]
                zt = statp.tile([128, FCW], F16, tag="zt", name="zt", bufs=2)
                nc.vector.tensor_tensor(out=zt, in0=yb, in1=rsb, op=Alu.mult)
                zt2 = statp.tile([128, FCW], F16, tag="zt2", name="zt2", bufs=4)
                nc.vector.tensor_tensor(out=zt2, in0=zt, in1=m2b,
                                        op=Alu.subtract)
                if flags["has_ln"]:
                    nc.vector.tensor_scalar(out=zt2, in0=zt2,
                                            scalar1=c_lngb[:, h, 0:1],
                                            scalar2=c_lngb[:, h, 1:2],
                                            op0=Alu.mult, op1=Alu.add)
                fts.append(zt2)
            return fts

        def gelu_block(fts_pairs):
            outs = []
            for zt in fts_pairs:
                ft = feat.tile([128, FCW], F16, tag="ft", name="ft", bufs=4)
                nc.scalar.activation(out=ft, in_=zt, func=Act.Gelu)
                outs.append(ft)
            return outs

        def om_block(sc, ftiles):
            """offset/mask logits for superchunk sc -> omT cols (bias folded)."""
            for half in range(2):
                ft0, ft1 = ftiles[half * H], ftiles[half * H + 1]
                for q in range(2):
                    po = psB.tile([128, 512], F32, tag="po", name="ps_om")
                    for h in range(H):
                        nc.tensor.matmul(
                            po[0:Q, :], lhsT=c_wom[:, h, :],
                            rhs=(ft0 if h == 0 else ft1)[:, q * 512:(q + 1) * 512],
                            start=(h == 0), stop=(h == H - 1),
                        )
                    col = sc * BCW + half * FCW + q * 512
                    nc.scalar.activation(out=omT[:, col:col + 512], in_=po[0:Q, :],
                                         func=Act.Identity, bias=c_bomd[0:Q, :])

        def band_block(sc):
            """band weights abf[l, tc, q=s*G+g] for superchunk sc."""
            om_l = bandp.tile([128, TC, Q], F16, tag="om_l", name="om_l", bufs=2)
            nc.sync.dma_start_transpose(
                out=om_l, in_=omT[:, sc * BCW:(sc + 1) * BCW])
            off3 = om_l[:, :, 0:24]
            fi = bandp.tile([128, TC, 24], I32, tag="fi", name="fi", bufs=1)
            nc.vector.tensor_copy(out=fi, in_=off3)
            ff3r = bandp.tile([128, TC, 24], F16, tag="ff3r", name="ff3r", bufs=2)
            nc.vector.tensor_copy(out=ff3r, in_=fi)
            # HW f16->i32 cast rounds to nearest; correct to floor
            fgt = bandp.tile([128, TC, 24], F16, tag="fgt", name="fgt", bufs=2)
            nc.vector.tensor_tensor(out=fgt, in0=ff3r, in1=off3, op=Alu.is_gt)
            ff3 = bandp.tile([128, TC, 24], F16, tag="ff3", name="ff3", bufs=2)
            nc.vector.tensor_tensor(out=ff3, in0=ff3r, in1=fgt, op=Alu.subtract)
            w1 = bandp.tile([128, TC, 24], F16, tag="w1", name="w1", bufs=2)
            nc.vector.tensor_tensor(out=w1, in0=off3, in1=ff3, op=Alu.subtract)
            mske = bandp.tile([128, TC, 24], F16, tag="mske", name="mske", bufs=2)
            nc.scalar.activation(out=mske, in_=om_l[:, :, 24:48], func=Act.Exp)
            mkv = mske.rearrange("p t (g k) -> p t g k", k=K)
            mko = bandp.tile([128, TC, G], F32, tag="mko", name="mko", bufs=2)
            nc.vector.tensor_reduce(out=mko, in_=mkv, axis=mybir.AxisListType.X,
                                    op=Alu.add)
            mks = bandp.tile([128, TC, G], F32, tag="mks", name="mks", bufs=2)
            nc.vector.reciprocal_approx_fast(out=mks, in_=mko)
            mskb = bandp.tile([128, TC, 24], F16, tag="mskb", name="mskb", bufs=2)
            mbc = bass.AP(tensor=mks.tensor, offset=mks.offset,
                          ap=[mks.ap[0], [G, TC], [1, G], [0, K]])
            nc.vector.tensor_tensor(out=mskb.rearrange("p t (g k) -> p t g k", k=K),
                                    in0=mkv, in1=mbc, op=Alu.mult)
            w1m = bandp.tile([128, TC, 24], F16, tag="w1m", name="w1m", bufs=2)
            nc.vector.tensor_tensor(out=w1m, in0=w1, in1=mskb, op=Alu.mult)
            b0m = bandp.tile([128, TC, 24], F16, tag="b0m", name="b0m", bufs=2)
            nc.vector.tensor_tensor(out=b0m, in0=mskb, in1=w1m, op=Alu.subtract)
            # zero-pad edge: w1 tap invalid at (l=0,ff3=2),(1,1),(L-2,4),(L-1,3)
            # (after b0m: the w0 tap keeps 1-frac even when w1 is dropped)
            if sc == 0 or sc == SC - 1:
                i = 0 if sc == 0 else 1
                tt = 0 if sc == 0 else TC - 1
                # et = (ff3 != edge) via TT-sub with a free-broadcast edge
                # column + immediate compare (AP-scalar TS is ~5.7us on DVE)
                et = bandp.tile([128, 24], F16, tag="et", name="et", bufs=2)
                ebc = bass.AP(tensor=cf.tensor, offset=cf.offset + 1 + i,
                              ap=[cf.ap[0], [0, 24]])
                nc.vector.tensor_tensor(out=et, in0=ff3[:, tt], in1=ebc,
                                        op=Alu.subtract)
                nc.vector.tensor_scalar(out=et, in0=et, scalar1=0.0,
                                        scalar2=None, op0=Alu.not_equal)
                nc.vector.tensor_tensor(out=w1m[:, tt], in0=w1m[:, tt],
                                        in1=et, op=Alu.mult)
            eq = {}
            for e in range(1, 5):  # ff3 in {1,2,3,4}
                t_ = bandp.tile([128, TC, 24], F16, tag="eq", name=f"eq{e}",
                                bufs=4)
                nc.vector.tensor_scalar(out=t_, in0=ff3, scalar1=float(e),
                                        scalar2=None, op0=Alu.is_equal)
                eq[e] = t_
            # abf padded to 128 q-cols: square xbar transpose (HW-proven form);
            # aT rows 40..127 are never read so pad content only needs init
            abf = bandp.tile([128, TC, 128], F16, tag="abf", name="abf", bufs=2)
            nc.vector.memset(abf[:, :, 40:128], 0.0)
            for s in range(NS):
                # cc = b0m*eq[s+1] + w1m*eq[s]  (eq[0], eq[5] empty)
                cc = bandp.tile([128, TC, 24], F16, tag="cc", name="cc", bufs=2)
                if s == 0:
                    nc.vector.tensor_tensor(out=cc, in0=b0m, in1=eq[1],
                                            op=Alu.mult)
                elif s == 4:
                    nc.vector.tensor_tensor(out=cc, in0=w1m, in1=eq[4],
                                            op=Alu.mult)
                else:
                    c2 = bandp.tile([128, TC, 24], F16, tag="cc", name="c2",
                                    bufs=2)
                    nc.vector.tensor_tensor(out=cc, in0=b0m, in1=eq[s + 1],
                                            op=Alu.mult)
                    nc.vector.tensor_tensor(out=c2, in0=w1m, in1=eq[s],
                                            op=Alu.mult)
                    nc.vector.tensor_add(cc, cc, c2)
                nc.vector.tensor_reduce(
                    out=abf[:, :, s * G:(s + 1) * G],
                    in_=cc.rearrange("p t (g k) -> p t g k", k=K),
                    axis=mybir.AxisListType.X, op=Alu.add)
            aT = bandp.tile([128, TC, 128], F16, tag="aT", name="aT", bufs=2)
            nc.sync.dma_start_transpose(
                out=aT, in_=abf.rearrange("p t q -> p (t q)"))
            return aT

        def xapply(sc, aT):
            """PE one-hot expansion (ws = E^T @ aT), scalar evac, fp16 apply
            (5 mults + 4 adds per hp at 2048), then the 32-pass out-proj."""
            l0 = sc * BCW
            shp = []
            for hp in range(H):
                ws = wsp.tile([128, NS, BCW], F16, tag="ws", name="ws", bufs=2)
                aTf = aT.rearrange("q t p -> q (t p)")
                for s in range(NS):
                    for half in range(2):
                        ps = psA.tile([128, FCW], F32, tag="ps", name="ps_ws")
                        for q in range(2):
                            nc.tensor.matmul(
                                ps[:, q * 512:(q + 1) * 512],
                                lhsT=c_E[:, hp * NS + s, :],
                                rhs=aTf[:, half * FCW + q * 512:
                                        half * FCW + (q + 1) * 512],
                                start=True, stop=True,
                            )
                        dst = ws[:, s, half * FCW:(half + 1) * FCW]
                        if s < 3:
                            nc.scalar.activation(out=dst, in_=ps, func=Act.Copy)
                        else:
                            nc.vector.tensor_copy(out=dst, in_=ps)
                samp = wsp.tile([128, BCW], F16, tag="samp", name="samp",
                                bufs=4)
                nc.vector.tensor_tensor(out=samp, in0=ws[:, 0, :],
                                        in1=xpT[hp][:, l0:l0 + BCW],
                                        op=Alu.mult)
                for s in range(1, NS):
                    tmp = wsp.tile([128, BCW], F16, tag="ztmp", name="ztmp",
                                   bufs=2)
                    nc.vector.tensor_tensor(
                        out=tmp, in0=ws[:, s, :],
                        in1=xpT[hp][:, l0 + s:l0 + s + BCW], op=Alu.mult)
                    nc.vector.tensor_add(samp, samp, tmp)
                shp.append(samp)
            ov = dram["out_d"].rearrange("(h p) l -> p h l", p=128)
            for co in range(H):
                ost = wsp.tile([128, BCW], F16, tag="ost", name="ost", bufs=2)
                for q in range(BCW // 512):
                    ps = psB.tile([128, 512], F32, tag="po", name="ps_out")
                    for ci in range(H):
                        nc.tensor.matmul(
                            ps,
                            lhsT=c_wout[:, ci, co * 128:(co + 1) * 128],
                            rhs=shp[ci][:, q * 512:(q + 1) * 512],
                            start=(ci == 0), stop=(ci == H - 1),
                        )
                    dst = ost[:, q * 512:(q + 1) * 512]
                    if flags["has_bout"]:
                        nc.scalar.activation(out=dst, in_=ps, func=Act.Identity,
                                             bias=c_bout[:, co:co + 1])
                    else:
                        nc.scalar.activation(out=dst, in_=ps, func=Act.Copy)
                nc.gpsimd.dma_start(out=ov[:, co, l0:l0 + BCW], in_=ost)

        # ---- pipeline ----
        pend = []
        for sc in range(SC):
            zts = []
            for half in range(2):
                zts.extend(feature_chunk(sc * 2 + half))
            fts = gelu_block(zts)
            om_block(sc, fts)
            if sc == 0:
                xpt_block()
            # band of THIS superchunk ahead of the previous one's apply on
            # the DVE queue, so this aT is ready and the PE's expansion can
            # start the moment it finishes the previous out-projection.
            aT = band_block(sc)
            if pend:
                xapply(*pend.pop())
            pend.append((sc, aT))
        xapply(*pend.pop())


def _prep_consts(inputs):
    f32 = np.float32
    W_in = np.asarray(inputs["W_in"], f32)
    W_off = np.asarray(inputs["W_off"], f32)
    W_mask = np.asarray(inputs["W_mask"], f32)
    W_out = np.asarray(inputs["W_out"], f32)
    conv_w = np.asarray(inputs["conv_w"], f32)[:, 0, :]      # [C, K]
    b_in = np.asarray(inputs["b_in"], f32)
    conv_b = np.asarray(inputs["conv_b"], f32)
    ln_g = np.asarray(inputs["ln_g"], f32)
    ln_b = np.asarray(inputs["ln_b"], f32)
    b_off = np.asarray(inputs["b_off"], f32)
    b_mask = np.asarray(inputs["b_mask"], f32)
    b_out = np.asarray(inputs["b_out"], f32)

    flags = {
        "has_bin": bool(np.any(b_in != 0)),
        "has_convb": bool(np.any(conv_b != 0)),
        "has_ln": bool(np.any(ln_g != 1) or np.any(ln_b != 0)),
        "has_bout": bool(np.any(b_out != 0)),
    }

    f16 = np.float16
    cm = {}
    parts = []
    parts.append(np.transpose(W_in.reshape(H, 128, C), (1, 0, 2)).reshape(128, -1))
    parts.append(np.transpose(W_out.reshape(H, 128, C), (1, 0, 2)).reshape(128, -1))
    # om weights: q = [off(g,k) 24 | mask(g,k) 24]  (reference (g,k) order)
    wom = np.zeros((C, Q), f32)
    wom[:, 0:24] = W_off
    wom[:, 24:48] = W_mask
    parts.append(np.transpose(wom.reshape(H, 128, Q), (1, 0, 2)).reshape(128, -1))
    dmats = np.zeros((K, H, 128, 128), f32)
    for j in range(K):
        for h in range(H):
            np.fill_diagonal(dmats[j, h], conv_w[h * 128:(h + 1) * 128, j])
    parts.append(np.transpose(dmats, (2, 0, 1, 3)).reshape(128, -1))
    parts.append(np.full((128, 128), 1.0 / C, f32))
    # one-hot expansion: E[q, i=(hp*NS+s), c] = 1 iff q = s*G + hp*4 + c//32
    Em = np.zeros((128, H * NS, 128), f32)
    for hp in range(H):
        for s in range(NS):
            for c in range(128):
                Em[s * G + hp * 4 + c // 32, hp * NS + s, c] = 1.0
    parts.append(Em.reshape(128, -1))
    cm["cbf"] = np.concatenate(parts, axis=1).astype(f16)

    f3 = np.zeros((128, 1 + 2 + 3 * H), f32)
    dg = np.tile(np.array([-1.0, 0.0, 1.0], f32), G)
    f3[0:24, 0] = b_off + dg + FB
    f3[24:48, 0] = b_mask
    f3[:, 1:3] = 99.0
    f3[0, 1] = 2.0      # l=0:   ff=-1 -> ff3=2
    f3[1, 1] = 1.0      # l=1:   ff=-2 -> ff3=1
    f3[126, 2] = 4.0    # l=L-2: ff=1  -> ff3=4
    f3[127, 2] = 3.0    # l=L-1: ff=0  -> ff3=3
    f3[:, 3:3 + H] = np.transpose(b_in.reshape(H, 128), (1, 0))
    f3[:, 3 + H:3 + 2 * H] = np.transpose(conv_b.reshape(H, 128), (1, 0))
    f3[:, 3 + 2 * H:3 + 3 * H] = np.transpose(b_out.reshape(H, 128), (1, 0))
    cm["cf3"] = f3
    if flags["has_ln"]:
        cm["lngb"] = np.transpose(
            np.stack([ln_g.reshape(H, 128), ln_b.reshape(H, 128)], axis=-1),
            (1, 0, 2)).astype(f16)
    return flags, cm


def kernel(**inputs):
    x = np.asarray(inputs["x"], np.float32)
    flags, cm = _prep_consts(inputs)

    key = tuple(sorted(flags.items()))
    if key not in _CACHE:
        _CACHE[key] = _build(flags)
    nc = _CACHE[key]

    in_maps = []
    for n in range(N):
        m = dict(cm)
        xt = np.zeros((C, L + 2), np.float16)
        xt[:, 1:L + 1] = x[n].T
        m["xt"] = xt
        in_maps.append(m)
    res = run_bass_kernel_spmd(nc, in_maps, core_ids=list(range(N)))
    out = np.stack([np.asarray(res.results[n]["out"], np.float32).T
                    for n in range(N)], axis=0)
    return out
